# revision 34
# baseline (speedup 1.0000x reference)
"""Multi-head attention (B=2, L=2048, D=1024, H=16, hd=64) on 8 TRN2 NeuronCores.

Sharding: tensor-parallel over heads — 2 heads per core. Each core computes
qkv projection for its heads, full attention for its (b, h) pairs, and a
partial output projection (w_proj columns for its heads); the host sums the
8 partial projections (f16 partials, accumulated in f32 host-side).

All matmuls are f16 operands with fp32 PSUM accumulation.

Dataflow per core (contraction dim always on partitions):
  qT,kT,vT [128, 4096] = w-slice.T @ xT        (transposed layout [j, t])
  vblk [128,160] per 128-key tile: [v_h0|1 .. v_h1|1] via PE transpose + copies
  scoresT  [tk, tq] = kT.T-slices @ qT-slices  (2 heads packed via tile_position)
  expT = exp(scoresT)                          (ACT, psum->sbuf f16)
  ou_h [65, tq] = [v_h|1].T @ expT             (fused attn@v + softmax denom)
  head = ou[v-rows] * bcast(1/ou[denom-row])   (DVE recip + GPSIMD bcast)
  outT_partial [e, t] = wprojT-slices.T @ headT  (f16 out, host sums partials)

Schedule: PE warm-up junk matmuls bridge the DMA prologue (HAM clock gate),
then a minimal prologue (k/q/v of t-tile 0 only), then ONE unified
128-group (b, tq, tk) attention stream with all remaining qkv groups and
all proj work as slot-mapped PE fillers; only the last t-tile's proj is in
the epilogue (with casts split DVE/ACT and per-quarter output DMAs).
Prologue DMAs are split across the sync+scalar HWDGE queues in
first-use order. Scores pairs + exps and the evacuation/normalization
DVE chain run under tc.high_priority() so the scheduler cannot park them
behind filler work; the e-ring is 8 deep so an exp's WAR on attnv(g-8)
never bites at tq boundaries. Filler units are consumed once per slot
(after the scores emission) and placed so every producer lands before its
consumer's slot, with proj units spread into PE-light odd slots well after
the producing tq's norm.
"""
import sys

if '/opt/trn_rl_repo' not in sys.path:
    sys.path.insert(0, '/opt/trn_rl_repo')

import numpy as np

B, L, D = 2, 2048, 1024
HEAD_DIM = 64
H = D // HEAD_DIM          # 16
NCORES = 8
HPC = H // NCORES          # 2 heads per core
T = B * L                  # 4096
KT = D // 128              # 8 contraction tiles for the projections
TT = T // 512              # 8 t-tiles of 512
TQ = L // 512              # 4 query tiles per batch
TK = L // 128              # 16 key tiles per batch


VTRANS_DMA = False


def _build_nc(reps: int = 1):
    import concourse.bacc as bacc
    import concourse.mybir as mybir
    import concourse.tile as tile
    from concourse.masks import make_identity
    from contextlib import nullcontext

    F32 = mybir.dt.float32
    F16 = mybir.dt.float16
    EXP = mybir.ActivationFunctionType.Exp

    nc = bacc.Bacc("TRN2", target_bir_lowering=False, debug=False,
                   num_devices=NCORES)
    xT_d = nc.dram_tensor("xT", [D, T], F16, kind="ExternalInput").ap()
    wqkvT_d = nc.dram_tensor("wqkvT", [D, 3 * 128], F16, kind="ExternalInput").ap()
    wprojT_d = nc.dram_tensor("wprojT", [128, D], F16, kind="ExternalInput").ap()
    outT_d = nc.dram_tensor("outT", [D, T], F16, kind="ExternalOutput").ap()

    with tile.TileContext(nc) as tc:
        with nc.allow_low_precision(reason="f16 matmul pipeline by design"), \
             tc.tile_pool(name="const", bufs=1) as cp, \
             tc.tile_pool(name="xt", bufs=2) as xp, \
             tc.tile_pool(name="exp", bufs=8) as ep, \
             tc.tile_pool(name="nrm", bufs=2) as np_, \
             tc.tile_pool(name="ps", bufs=2, space="PSUM") as ps:

            ones_f = cp.tile([128, 1], F32, tag="onesf")
            nc.gpsimd.memset(ones_f[:], 1.0)
            # f16 ones row [1, 64]: stationary operand of the PE broadcast
            # matmul (out[64, 512] = onesrow.T @ r16)
            onesrow = cp.tile([1, 64], F16, tag="onesrow")
            nc.gpsimd.memset(onesrow[:], 1.0)
            if not VTRANS_DMA:
                ident_f = cp.tile([128, 128], F32, tag="identf")
                make_identity(nc, ident_f[:])
                ident = cp.tile([128, 128], F16, tag="ident")
                nc.vector.tensor_copy(ident[:], ident_f[:])

            # PE warm-up: the HAM clock gate keeps the PE at 1.2 GHz until
            # ~3.4us of sustained activity. Junk matmuls bridge the DMA
            # prologue so the real stream starts (nearly) warm.
            junkw = cp.tile([128, 256], F16, tag="junkw")
            nc.gpsimd.memset(junkw[:], 0.0)

            # weights: three per-part tiles so the first matmul only waits on
            # its own 256KB slice
            w_p = [cp.tile([128, KT, 128], F16, tag=f"w{p}", name=f"w{p}")
                   for p in range(3)]

            def dma_w(p, c0, n=4):
                # weight transfers ride the scalar (ACT) HWDGE queue: ACT is
                # idle until the first scores land, and this keeps the sync
                # queue free for the x tiles. c0/n select a chunk range.
                nc.scalar.dma_start(
                    w_p[p][:, c0:c0 + n, :],
                    wqkvT_d[:, p * 128:(p + 1) * 128].rearrange(
                        "(k p) j -> p k j", p=128)[:, c0:c0 + n, :])
            wp_t = cp.tile([128, 1024], F16, tag="wp")

            # persistent activations
            qT = cp.tile([128, T], F16, tag="qT")
            kTt = cp.tile([128, T], F16, tag="kTt")
            vT = cp.tile([128, T], F16, tag="vT")
            headT = cp.tile([128, T], F16, tag="headT")
            # vblk layout (160 cols): [v_h0 0:64 | ones 64 | pad | v_h1
            # 80:144 | ones 144 | pad]; lhsT h0 = [:, 0:65], h1 = [:, 80:145],
            # both -> ou = [A rows 0-63; Z row 64]. Both v halves are written
            # by ONE DMA-XBAR transpose via a [128, 2, 64] strided view
            # (dim1 stride 80 cols = 160B, 32B-aligned).
            vblk = [[cp.tile([128, 160], F16, tag=f"vb{b}_{tk}",
                             name=f"vb{b}_{tk}") for tk in range(TK)]
                    for b in range(B)]
            for b in range(B):
                for tk in range(TK):
                    # one strided memset covers both ones columns (64, 144);
                    # gpsimd keeps this startup work off the DVE
                    oc = vblk[b][tk][:, 0:160].rearrange(
                        "p (two c) -> p two c", c=80)[:, :, 64:65]
                    nc.gpsimd.memset(oc, 1.0)

            # x: all tiles prefetched, two DMAs per tile (matching the two
            # 4-chunk matmul halves) so consumers wait only on their half and
            # the sync engine isn't clogged with dispatches
            xts = [xp.tile([128, KT, 512], F16, tag="xt", name=f"xt{t}",
                           bufs=8) for t in range(TT)]

            def dma_x(t, h, eng, n=4):
                c0 = int(4 * h)
                eng.dma_start(
                    xts[t][:, c0:c0 + n, :],
                    xT_d[:, t * 512:t * 512 + 512].rearrange(
                        "(k p) t -> p k t", p=128)[:, c0:c0 + n, :])

            # Prologue transfers split across the two HWDGE dispatchers:
            # weights on scalar, x on sync, both ordered to match the
            # prologue group order (k0, q0, v0). Early transfers ride HBM
            # contention from all 8 cores prefetching at once, so the very
            # first chunks are split finer to get the first matmul going.
            dma_w(1, 0, 2)        # wk chunks 0-1 (scalar queue)
            dma_x(0, 0, nc.sync, 2)   # xt0 chunks 0-1 (sync queue)
            dma_w(1, 2, 2)        # wk chunks 2-3
            dma_x(0, 0.5, nc.sync, 2)  # xt0 chunks 2-3
            dma_w(1, 4, 4)        # wk h1
            dma_x(0, 1, nc.sync)  # xt0 h1
            dma_w(0, 0, 4)        # wq h0
            dma_w(0, 4, 4)        # wq h1
            dma_w(2, 0, 4)        # wv h0
            dma_w(2, 4, 4)        # wv h1
            dma_x(1, 0, nc.sync)
            dma_x(1, 1, nc.sync)
            for t in (2, 3):
                dma_x(t, 0, nc.sync)
                dma_x(t, 1, nc.sync)
            nc.sync.dma_start(wp_t[:], wprojT_d[:, :])
            # x t4-7 dispatches are deferred into early phase-1 filler slots
            # (they are not needed until ~40us in).

            # warm the ACT exp table (queued on scalar after the w
            # dispatches; done well before the first real scores)
            actw = cp.tile([128, 1], F32, tag="actw")
            nc.scalar.activation(actw[:], ones_f[:], EXP)

            # PE warm-up stream: ~16 junk N=256 matmuls ~= 3.4us at the cold
            # 1.2 GHz clock, so the HAM un-throttles by the time the real
            # pipeline is flowing
            jp = ps.tile([128, 512], F32, tag="sm", name="jp", bufs=2)
            for _ in range(10):
                nc.tensor.matmul(jp[:, 0:256], junkw[:, 0:128],
                                 junkw[:, 0:256], start=True, stop=True)

            def emit_A_group_gen(t, part):
                """one qkv projection group: dest[:, t-tile] for q/k/v part;
                generator yielding mid-group for finer interleaving"""
                dest = (qT, kTt, vT)[part]
                xt = xts[t]
                s = ps.tile([128, 512], F32, tag="sm", name="sA", bufs=2)
                for half in range(2):
                    for k in range(4 * half, 4 * half + 4):
                        nc.tensor.matmul(
                            s[:],
                            w_p[part][:, k, :],
                            xt[:, k, :],
                            start=(k == 0), stop=(k == KT - 1))
                    yield None
                # the evacuation gates scores/attnv of later slots — keep it
                # ahead of bulk DVE work (proj casts) in the queue
                with tc.high_priority():
                    nc.vector.tensor_copy(dest[:, t * 512:t * 512 + 512],
                                          s[:])
                if part == 2:
                    # v landed: transpose this t-tile's 4 key blocks into
                    # vblk — via DMA XBAR (pure DMA, no PE slot) or PE
                    b, tq = divmod(t, TQ)
                    for i in range(4):
                        tk = tq * 4 + i
                        c0 = b * L + tk * 128
                        vb = vblk[b][tk]
                        if VTRANS_DMA:
                            # XBAR transpose: out partition j <- src free j,
                            # out free col <- src partition. One DMA per head
                            # (a strided multi-dim dst is treated as extra
                            # partition dims by dma_start_transpose, so the
                            # two 64-col halves need separate transfers).
                            nc.sync.dma_start(vb[:, 0:64],
                                              vT[0:64, c0:c0 + 128],
                                              transpose=True)
                            nc.sync.dma_start(vb[:, 80:144],
                                              vT[64:128, c0:c0 + 128],
                                              transpose=True)
                        else:
                            p32 = ps.tile([128, 512], F32, tag="sm",
                                          name="ptr", bufs=2)
                            pt = p32.bitcast(F16)
                            nc.tensor.transpose(pt[:, 0:128],
                                                vT[:, c0:c0 + 128], ident[:])
                            nc.vector.tensor_copy(vb[:, 0:64],
                                                  pt[0:128, 0:64])
                            nc.vector.tensor_copy(vb[:, 80:144],
                                                  pt[0:128, 64:128])

            def emit_proj(t, e2, tail=False):
                # one unit = quarter of the output rows for one t-tile:
                # 4 matmuls, 4 casts, one 512KB DMA (fewer dispatches/sems).
                # tail=True (final t-tile): split casts across DVE+ACT and
                # DMA per quarter on alternating queues to shorten the
                # epilogue critical chain.
                po = xp.tile([128, 4, 512], F16, tag="po", name="po", bufs=3)
                for quar in range(4):
                    e8 = e2 * 4 + quar
                    pp = ps.tile([128, 512], F32, tag="sm", name="pp", bufs=2)
                    nc.tensor.matmul(pp[:],
                                     wp_t[:, e8 * 128:(e8 + 1) * 128],
                                     headT[:, t * 512:t * 512 + 512],
                                     start=True, stop=True)
                    if tail and quar % 2 == 1:
                        nc.scalar.copy(po[:, quar, :], pp[:])
                    else:
                        nc.vector.tensor_copy(po[:, quar, :], pp[:])
                    if tail:
                        eng = nc.sync if quar % 2 == 0 else nc.scalar
                        eng.dma_start(
                            outT_d[e8 * 128:(e8 + 1) * 128,
                                   t * 512:t * 512 + 512].rearrange(
                                "(one p) t -> p one t", p=128),
                            po[:, quar:quar + 1, :])
                if not tail:
                    nc.sync.dma_start(
                        outT_d[e2 * 512:(e2 + 1) * 512,
                               t * 512:t * 512 + 512].rearrange(
                            "(four p) t -> p four t", p=128),
                        po[:])

            def emit_attn(fillers, pe_bcast_last=True):
                """attention for BOTH batches as one continuous 128-group
                stream; fillers: iterator of callables used to keep PE dense
                while ACT works through the exps"""
                # Software-pipelined over ALL (b, tq, tk) tiles: the
                # scores/exp stream runs 2 slots ahead of the attnv stream
                # continuously across tq AND batch boundaries, so the
                # pipeline never drains mid-kernel (a drain also triggered
                # HAM re-throttles).
                NTOT = B * TQ * TK
                ous = {}
                epipe = {}
                for g in range(NTOT + 2):
                    # scores go FIRST in the slot: at tq boundaries the attnv
                    # stream blocks on the ou-ring until the norm evacuation
                    # frees it — with scores ahead in the PE FIFO, the
                    # exp pipeline keeps flowing through the stall.
                    if g < NTOT:
                        b, rem = divmod(g, TQ * TK)
                        tq, tk = divmod(rem, TK)
                        q0 = b * L + tq * 512
                        k0 = b * L + tk * 128
                        s = ps.tile([128, 1024], F32, tag="sc", name="sB",
                                     bufs=2)
                        # high_priority pins the pair + exp at the head of
                        # the scheduler's ready heap: without it, the
                        # scheduler sometimes splits the tile-position pair
                        # around filler matmuls, delaying the exp ~2us
                        with tc.high_priority():
                            nc.tensor.matmul(s[:, 0:512],
                                             kTt[0:64, k0:k0 + 128],
                                             qT[0:64, q0:q0 + 512],
                                             start=True, stop=True,
                                             tile_position=(0, 0))
                            nc.tensor.matmul(s[:, 512:1024],
                                             kTt[64:128, k0:k0 + 128],
                                             qT[64:128, q0:q0 + 512],
                                             start=True, stop=True,
                                             tile_position=(64, 0))
                            e = ep.tile([128, 1024], F16, tag="e", name="e")
                            nc.scalar.activation(e[:], s[:], EXP)
                        epipe[g] = e
                    if fillers is not None:
                        try:
                            next(fillers)()
                        except StopIteration:
                            fillers = None
                    ga = g - 2
                    if ga < 0:
                        continue
                    b_a, rem_a = divmod(ga, TQ * TK)
                    aq, aj = divmod(rem_a, TK)
                    gq = ga // TK      # global tq index (0..B*TQ-1)
                    if aj == 0:
                        ous[gq] = [ps.tile([65, 512], F32, tag="outU",
                                           name=f"ou{h}", bufs=2)
                                   for h in range(2)]
                    ou = ous[gq]
                    e = epipe.pop(ga)
                    nc.tensor.matmul(ou[0][:], vblk[b_a][aj][:, 0:65],
                                     e[:, 0:512],
                                     start=(aj == 0), stop=(aj == TK - 1))
                    nc.tensor.matmul(ou[1][:], vblk[b_a][aj][:, 80:145],
                                     e[:, 512:1024],
                                     start=(aj == 0), stop=(aj == TK - 1))
                    if aj != TK - 1:
                        continue
                    # tq aq complete: evacuate ou to SBUF so the PSUM banks
                    # free quickly; normalization runs from SBUF off the
                    # critical path. ou = [A rows 0-63; Z row 64] per head.
                    # Z rows + reciprocals first so the gpsimd broadcasts
                    # start while the DVE still evacuates the A rows.
                    ou = ous.pop(gq)
                    qa = b_a * L + aq * 512
                    pe_bc = pe_bcast_last and ga == NTOT - 1
                    # The four ou evacuation copies (rs = Z rows, ouS = A
                    # rows) come FIRST and elevated: the ou ring — and with
                    # it the next tq's attnv — frees as soon as they run.
                    # The slow single-partition reciprocals (~0.7us each),
                    # broadcasts and muls follow off the critical path.
                    rss, ouSs = [], []
                    for h in range(2):
                        rs = np_.tile([1, 512], F32, tag=f"rs{h}",
                                      name=f"rs{h}")
                        if pe_bc:
                            # ACT is idle at stream end: off-load the copies
                            # so the DVE queue only carries recip+mul on the
                            # epilogue critical path
                            nc.scalar.copy(rs[:], ou[h][64:65, :])
                        else:
                            with tc.high_priority():
                                nc.vector.tensor_copy(rs[:], ou[h][64:65, :])
                        rss.append(rs)
                    for h in range(2):
                        ouS = np_.tile([64, 512], F32, tag=f"ouS{h}",
                                       name=f"ouS{h}")
                        if pe_bc:
                            nc.scalar.copy(ouS[:], ou[h][0:64, :])
                        else:
                            with tc.high_priority():
                                nc.vector.tensor_copy(ouS[:], ou[h][0:64, :])
                        ouSs.append(ouS)
                    rrs = []
                    for h in range(2):
                        r = np_.tile([1, 512], F32, tag=f"r{h}", name=f"r{h}")
                        nc.vector.reciprocal_approx_fast(r[:], rss[h])
                        if pe_bc:
                            # last tq of the run: broadcast 1/Z on the PE —
                            # the scores PSUM ring is free here, the gpsimd
                            # ucode broadcast (1.8us) sits on the epilogue
                            # critical path, and the matmul keeps HAM warm
                            # for the final proj
                            r16 = np_.tile([1, 512], F16, tag=f"r16{h}",
                                           name=f"r16{h}")
                            nc.scalar.copy(r16[:], r[:])
                            bcp = ps.tile([128, 1024], F32, tag="sc",
                                          name="bcp", bufs=2)
                            nc.tensor.matmul(bcp[0:64, 0:512], onesrow[:],
                                             r16[:], start=True, stop=True)
                            rrs.append(bcp[0:64, 0:512])
                        else:
                            bc = np_.tile([64, 512], F32, tag=f"bc{h}",
                                          name=f"bc{h}")
                            nc.gpsimd.partition_broadcast(bc[:], r[:])
                            rrs.append(bc[:])
                    for h in range(2):
                        nc.vector.tensor_mul(
                            headT[h * 64:(h + 1) * 64, qa:qa + 512],
                            ouSs[h], rrs[h])
                if fillers is not None:
                    for f in fillers:
                        f()

            def run_group(t, part):
                for _ in emit_A_group_gen(t, part):
                    pass

            def group_units2(t, part):
                # compressed 2-unit split (keeps the sA ring hold short and
                # the group's completion early)
                gen = emit_A_group_gen(t, part)
                return [lambda g=gen: next(g, None),
                        lambda g=gen: list(g)]

            def group_units3(t, part):
                gen = emit_A_group_gen(t, part)
                return [lambda g=gen: next(g, None),
                        lambda g=gen: next(g, None),
                        lambda g=gen: list(g)]

            def place(slots, at, units):
                for i, u in zip(at, units):
                    assert slots[i] is None, f"slot {i} already taken"
                    slots[i] = u

            with (tc.For_i(0, reps, 1) if reps > 1 else nullcontext()):
                # Minimal prologue: only what scores(0)/attnv(0) need (k, q,
                # v of t-tile 0). The first exp is then ~3 groups after the
                # first DMA lands instead of 5 — and none of the remaining
                # qkv evacuations sit ahead of it in the in-order DVE queue.
                for t, part in ((0, 1), (0, 0), (0, 2)):
                    run_group(t, part)
                # One unified 128-group attention stream over both batches.
                # Filler slot map (consumed at slot START; a group's last
                # unit must land before its consumer):
                #  b0: scores(tk) at slot tk reads kTt/qT, attnv(tk) at
                #      slot tk+2 reads vblk; qN by slot 16*N.
                #  b1: scores at slot 64+tk (kTt t4-7 map to b1 tk0-15),
                #      attnv at 66+tk; q4 by 64, q5 by 80, q6 by 96,
                #      q7 by 112.
                #  proj(t, e2) units go in PE-light odd slots well after the
                #  producing tq's norm, spread out so their DVE casts never
                #  chain (the sm ring serializes a bunched proj stream at
                #  cast rate).
                def fill():
                    def disp(ts):
                        for t in ts:
                            dma_x(t, 0, nc.sync)
                            dma_x(t, 1, nc.sync)
                    slots = [None] * 130
                    place(slots, (0, 1), group_units2(1, 1))
                    place(slots, (2, 3), group_units2(1, 2))
                    place(slots, (4, 6), group_units2(2, 1))
                    place(slots, (5, 7), group_units2(2, 2))
                    place(slots, (8, 9), group_units2(3, 1))
                    place(slots, (10, 11), group_units2(3, 2))
                    place(slots, (12, 13, 14), group_units3(1, 0))
                    place(slots, (16, 18, 20), group_units3(4, 1))
                    place(slots, (22, 24, 26), group_units3(2, 0))
                    place(slots, (28, 30, 32), group_units3(4, 2))
                    place(slots, (34, 36, 38), group_units3(5, 1))
                    place(slots, (40, 42, 44), group_units3(3, 0))
                    place(slots, (46, 48, 50), group_units3(5, 2))
                    place(slots, (52, 54, 56), group_units3(4, 0))
                    place(slots, (58, 60), group_units2(6, 1))
                    place(slots, (59, 61), group_units2(6, 2))
                    place(slots, (63, 65), group_units2(7, 1))
                    place(slots, (64, 66), group_units2(7, 2))
                    place(slots, (68, 70, 72), group_units3(5, 0))
                    place(slots, (74, 76, 78), group_units3(6, 0))
                    place(slots, (82, 84, 86), group_units3(7, 0))
                    proj_at = {0: (47, 49), 1: (51, 53), 2: (55, 57),
                               3: (69, 71), 4: (88, 90), 5: (100, 102),
                               6: (116, 118)}
                    for t, (a, b2) in proj_at.items():
                        place(slots, (a, b2),
                              [lambda tt=t: emit_proj(tt, 0),
                               lambda tt=t: emit_proj(tt, 1)])
                    extras = {0: lambda: disp((4, 5)), 1: lambda: disp((6, 7))}
                    for i, u in enumerate(slots):
                        e = extras.get(i)
                        if e is not None:
                            yield (lambda ee=e, uu=u:
                                   (ee(), uu() if uu else None) and None)
                        else:
                            yield u if u is not None else (lambda: None)
                emit_attn(fill())
                # epilogue: only the last t-tile's proj remains
                for e2 in range(2):
                    emit_proj(7, e2, tail=True)

    nc.compile()
    return nc

_CACHE = {}


def _get_nc(reps: int = 1):
    key = reps
    if key not in _CACHE:
        _CACHE[key] = _build_nc(reps)
    return _CACHE[key]


def _make_in_maps(x, w_qkv, w_proj):
    xT = np.ascontiguousarray(x.reshape(T, D).T).astype(np.float16)
    in_maps = []
    for c in range(NCORES):
        j0 = c * 128
        wq = w_qkv[j0:j0 + 128] * 0.125          # fold attention scale into q
        wk = w_qkv[D + j0:D + j0 + 128]
        wv = w_qkv[2 * D + j0:2 * D + j0 + 128]
        wqkvT = np.ascontiguousarray(
            np.concatenate([wq, wk, wv], axis=0).T).astype(np.float16)
        wprojT = np.ascontiguousarray(w_proj[:, j0:j0 + 128].T).astype(np.float16)
        in_maps.append({"xT": xT, "wqkvT": wqkvT, "wprojT": wprojT})
    return in_maps


def _numpy_reference(x, mask, w_qkv, w_proj):
    x64 = x.astype(np.float64)
    qkv = (x64 @ w_qkv.T.astype(np.float64)).reshape(B, L, 3, H, HEAD_DIM)
    qkv = qkv.transpose(2, 0, 3, 1, 4)
    q, k, v = qkv[0], qkv[1], qkv[2]
    attn = np.einsum('bhqd,bhkd->bhqk', q, k) * (HEAD_DIM ** -0.5)
    attn = np.where(mask[:, None, :, :], attn, -np.inf)
    attn = attn - attn.max(axis=-1, keepdims=True)
    attn = np.exp(attn)
    attn = attn / attn.sum(axis=-1, keepdims=True)
    out = np.einsum('bhqk,bhkd->bhqd', attn, v)
    out = out.transpose(0, 2, 1, 3).reshape(B, L, D)
    return (out @ w_proj.T.astype(np.float64)).astype(np.float32)


def kernel(x, mask, w_qkv, w_proj):
    x = np.asarray(x)
    mask = np.asarray(mask)
    w_qkv = np.asarray(w_qkv)
    w_proj = np.asarray(w_proj)
    if not mask.all():
        # spec guarantees an all-ones mask; keep a correct fallback anyway
        return _numpy_reference(x, mask, w_qkv, w_proj)

    from concourse import bass_utils
    nc = _get_nc()
    in_maps = _make_in_maps(x, w_qkv, w_proj)
    res = bass_utils.run_bass_kernel_spmd(nc, in_maps,
                                          core_ids=list(range(NCORES)))
    acc = np.zeros((D, T), np.float32)
    for c in range(NCORES):
        acc += res.results[c]["outT"].astype(np.float32)
    return np.ascontiguousarray(acc.T).reshape(B, L, D)


if __name__ == "__main__":
    rng = np.random.default_rng(0)
    x = rng.standard_normal((B, L, D)).astype(np.float32)
    mask = np.ones((B, L, L), bool)
    w_qkv = (rng.standard_normal((3 * D, D)) * D ** -0.5).astype(np.float32)
    w_proj = (rng.standard_normal((D, D)) * D ** -0.5).astype(np.float32)
    out = kernel(x, mask, w_qkv, w_proj)
    exp = _numpy_reference(x, mask, w_qkv, w_proj)
    err = np.abs(out - exp).max() / np.abs(exp).max()
    print("rel err vs fp64 numpy reference:", err)



# revision 35
# speedup vs baseline: 1.0060x; 1.0060x over previous
"""Multi-head attention (B=2, L=2048, D=1024, H=16, hd=64) on 8 TRN2 NeuronCores.

Sharding: tensor-parallel over heads — 2 heads per core. Each core computes
qkv projection for its heads, full attention for its (b, h) pairs, and a
partial output projection (w_proj columns for its heads); the host sums the
8 partial projections (f16 partials, accumulated in f32 host-side).

All matmuls are f16 operands with fp32 PSUM accumulation.

Dataflow per core (contraction dim always on partitions):
  qT,kT,vT [128, 4096] = w-slice.T @ xT        (transposed layout [j, t])
  vblk [128,160] per 128-key tile: [v_h0|1 .. v_h1|1] via PE transpose + copies
  scoresT  [tk, tq] = kT.T-slices @ qT-slices  (2 heads packed via tile_position)
  expT = exp(scoresT)                          (ACT, psum->sbuf f16)
  ou_h [65, tq] = [v_h|1].T @ expT             (fused attn@v + softmax denom)
  head = ou[v-rows] * bcast(1/ou[denom-row])   (DVE recip + GPSIMD bcast)
  outT_partial [e, t] = wprojT-slices.T @ headT  (f16 out, host sums partials)

Schedule: PE warm-up junk matmuls bridge the DMA prologue (HAM clock gate),
then a minimal prologue (k/q/v of t-tile 0 only), then ONE unified
128-group (b, tq, tk) attention stream with all remaining qkv groups and
all proj work as slot-mapped PE fillers; only the last t-tile's proj is in
the epilogue (with casts split DVE/ACT and per-quarter output DMAs).
Prologue DMAs are split across the sync+scalar HWDGE queues in
first-use order. Scores pairs + exps and the evacuation/normalization
DVE chain run under tc.high_priority() so the scheduler cannot park them
behind filler work; the e-ring is 8 deep so an exp's WAR on attnv(g-8)
never bites at tq boundaries. Filler units are consumed once per slot
(after the scores emission) and placed so every producer lands before its
consumer's slot, with proj units spread into PE-light odd slots well after
the producing tq's norm.
"""
import sys

if '/opt/trn_rl_repo' not in sys.path:
    sys.path.insert(0, '/opt/trn_rl_repo')

import numpy as np

B, L, D = 2, 2048, 1024
HEAD_DIM = 64
H = D // HEAD_DIM          # 16
NCORES = 8
HPC = H // NCORES          # 2 heads per core
T = B * L                  # 4096
KT = D // 128              # 8 contraction tiles for the projections
TT = T // 512              # 8 t-tiles of 512
TQ = L // 512              # 4 query tiles per batch
TK = L // 128              # 16 key tiles per batch


VTRANS_DMA = False


def _build_nc(reps: int = 1):
    import concourse.bacc as bacc
    import concourse.mybir as mybir
    import concourse.tile as tile
    from concourse.masks import make_identity
    from contextlib import nullcontext

    F32 = mybir.dt.float32
    F16 = mybir.dt.float16
    EXP = mybir.ActivationFunctionType.Exp

    nc = bacc.Bacc("TRN2", target_bir_lowering=False, debug=False,
                   num_devices=NCORES)
    xT_d = nc.dram_tensor("xT", [D, T], F16, kind="ExternalInput").ap()
    wqkvT_d = nc.dram_tensor("wqkvT", [D, 3 * 128], F16, kind="ExternalInput").ap()
    wprojT_d = nc.dram_tensor("wprojT", [128, D], F16, kind="ExternalInput").ap()
    outT_d = nc.dram_tensor("outT", [D, T], F16, kind="ExternalOutput").ap()

    with tile.TileContext(nc) as tc:
        with nc.allow_low_precision(reason="f16 matmul pipeline by design"), \
             tc.tile_pool(name="const", bufs=1) as cp, \
             tc.tile_pool(name="xt", bufs=2) as xp, \
             tc.tile_pool(name="exp", bufs=8) as ep, \
             tc.tile_pool(name="nrm", bufs=2) as np_, \
             tc.tile_pool(name="ps", bufs=2, space="PSUM") as ps:

            ones_f = cp.tile([128, 1], F32, tag="onesf")
            nc.gpsimd.memset(ones_f[:], 1.0)
            # f16 ones row [1, 64]: stationary operand of the PE broadcast
            # matmul (out[64, 512] = onesrow.T @ r16)
            onesrow = cp.tile([1, 64], F16, tag="onesrow")
            nc.gpsimd.memset(onesrow[:], 1.0)
            if not VTRANS_DMA:
                ident_f = cp.tile([128, 128], F32, tag="identf")
                make_identity(nc, ident_f[:])
                ident = cp.tile([128, 128], F16, tag="ident")
                nc.vector.tensor_copy(ident[:], ident_f[:])

            # PE warm-up: the HAM clock gate keeps the PE at 1.2 GHz until
            # ~3.4us of sustained activity. Junk matmuls bridge the DMA
            # prologue so the real stream starts (nearly) warm.
            junkw = cp.tile([128, 256], F16, tag="junkw")
            nc.gpsimd.memset(junkw[:], 0.0)

            # weights: three per-part tiles so the first matmul only waits on
            # its own 256KB slice
            w_p = [cp.tile([128, KT, 128], F16, tag=f"w{p}", name=f"w{p}")
                   for p in range(3)]

            def dma_w(p, c0, n=4):
                # weight transfers ride the scalar (ACT) HWDGE queue: ACT is
                # idle until the first scores land, and this keeps the sync
                # queue free for the x tiles. c0/n select a chunk range.
                nc.scalar.dma_start(
                    w_p[p][:, c0:c0 + n, :],
                    wqkvT_d[:, p * 128:(p + 1) * 128].rearrange(
                        "(k p) j -> p k j", p=128)[:, c0:c0 + n, :])
            wp_t = cp.tile([128, 1024], F16, tag="wp")

            # persistent activations
            qT = cp.tile([128, T], F16, tag="qT")
            kTt = cp.tile([128, T], F16, tag="kTt")
            vT = cp.tile([128, T], F16, tag="vT")
            headT = cp.tile([128, T], F16, tag="headT")
            # vblk layout (160 cols): [v_h0 0:64 | ones 64 | pad | v_h1
            # 80:144 | ones 144 | pad]; lhsT h0 = [:, 0:65], h1 = [:, 80:145],
            # both -> ou = [A rows 0-63; Z row 64]. Both v halves are written
            # by ONE DMA-XBAR transpose via a [128, 2, 64] strided view
            # (dim1 stride 80 cols = 160B, 32B-aligned).
            vblk = [[cp.tile([128, 160], F16, tag=f"vb{b}_{tk}",
                             name=f"vb{b}_{tk}") for tk in range(TK)]
                    for b in range(B)]
            for b in range(B):
                for tk in range(TK):
                    # one strided memset covers both ones columns (64, 144);
                    # gpsimd keeps this startup work off the DVE
                    oc = vblk[b][tk][:, 0:160].rearrange(
                        "p (two c) -> p two c", c=80)[:, :, 64:65]
                    nc.gpsimd.memset(oc, 1.0)

            # x: all tiles prefetched, two DMAs per tile (matching the two
            # 4-chunk matmul halves) so consumers wait only on their half and
            # the sync engine isn't clogged with dispatches
            xts = [xp.tile([128, KT, 512], F16, tag="xt", name=f"xt{t}",
                           bufs=8) for t in range(TT)]

            def dma_x(t, h, eng, n=4):
                c0 = int(4 * h)
                eng.dma_start(
                    xts[t][:, c0:c0 + n, :],
                    xT_d[:, t * 512:t * 512 + 512].rearrange(
                        "(k p) t -> p k t", p=128)[:, c0:c0 + n, :])

            # Prologue transfers split across the two HWDGE dispatchers:
            # weights on scalar, x on sync, both ordered to match the
            # prologue group order (k0, q0, v0). Early transfers ride HBM
            # contention from all 8 cores prefetching at once, so the very
            # first chunks are split finer to get the first matmul going.
            dma_w(1, 0, 2)        # wk chunks 0-1 (scalar queue)
            dma_x(0, 0, nc.sync, 2)   # xt0 chunks 0-1 (sync queue)
            dma_w(1, 2, 2)        # wk chunks 2-3
            dma_x(0, 0.5, nc.sync, 2)  # xt0 chunks 2-3
            dma_w(1, 4, 4)        # wk h1
            dma_x(0, 1, nc.sync)  # xt0 h1
            dma_w(0, 0, 4)        # wq h0
            dma_w(0, 4, 4)        # wq h1
            dma_w(2, 0, 4)        # wv h0
            dma_w(2, 4, 4)        # wv h1
            dma_x(1, 0, nc.sync)
            dma_x(1, 1, nc.sync)
            for t in (2, 3):
                dma_x(t, 0, nc.sync)
                dma_x(t, 1, nc.sync)
            nc.sync.dma_start(wp_t[:], wprojT_d[:, :])
            # x t4-7 dispatches are deferred into early phase-1 filler slots
            # (they are not needed until ~40us in).

            # warm the ACT exp table (queued on scalar after the w
            # dispatches; done well before the first real scores)
            actw = cp.tile([128, 1], F32, tag="actw")
            nc.scalar.activation(actw[:], ones_f[:], EXP)

            # PE warm-up stream: ~16 junk N=256 matmuls ~= 3.4us at the cold
            # 1.2 GHz clock, so the HAM un-throttles by the time the real
            # pipeline is flowing
            jp = ps.tile([128, 512], F32, tag="sm", name="jp", bufs=2)
            for _ in range(10):
                nc.tensor.matmul(jp[:, 0:256], junkw[:, 0:128],
                                 junkw[:, 0:256], start=True, stop=True)

            def emit_A_group_gen(t, part):
                """one qkv projection group: dest[:, t-tile] for q/k/v part;
                generator yielding mid-group for finer interleaving"""
                dest = (qT, kTt, vT)[part]
                xt = xts[t]
                s = ps.tile([128, 512], F32, tag="sm", name="sA", bufs=2)
                for half in range(2):
                    for k in range(4 * half, 4 * half + 4):
                        nc.tensor.matmul(
                            s[:],
                            w_p[part][:, k, :],
                            xt[:, k, :],
                            start=(k == 0), stop=(k == KT - 1))
                    yield None
                # the evacuation gates scores/attnv of later slots — keep it
                # ahead of bulk DVE work (proj casts) in the queue
                with tc.high_priority():
                    nc.vector.tensor_copy(dest[:, t * 512:t * 512 + 512],
                                          s[:])
                if part == 2:
                    # v landed: transpose this t-tile's 4 key blocks into
                    # vblk — via DMA XBAR (pure DMA, no PE slot) or PE
                    b, tq = divmod(t, TQ)
                    for i in range(4):
                        tk = tq * 4 + i
                        c0 = b * L + tk * 128
                        vb = vblk[b][tk]
                        if VTRANS_DMA:
                            # XBAR transpose: out partition j <- src free j,
                            # out free col <- src partition. One DMA per head
                            # (a strided multi-dim dst is treated as extra
                            # partition dims by dma_start_transpose, so the
                            # two 64-col halves need separate transfers).
                            nc.sync.dma_start(vb[:, 0:64],
                                              vT[0:64, c0:c0 + 128],
                                              transpose=True)
                            nc.sync.dma_start(vb[:, 80:144],
                                              vT[64:128, c0:c0 + 128],
                                              transpose=True)
                        else:
                            p32 = ps.tile([128, 512], F32, tag="sm",
                                          name="ptr", bufs=2)
                            pt = p32.bitcast(F16)
                            nc.tensor.transpose(pt[:, 0:128],
                                                vT[:, c0:c0 + 128], ident[:])
                            nc.vector.tensor_copy(vb[:, 0:64],
                                                  pt[0:128, 0:64])
                            nc.vector.tensor_copy(vb[:, 80:144],
                                                  pt[0:128, 64:128])

            def emit_proj(t, e2, tail=False):
                # one unit = quarter of the output rows for one t-tile:
                # 4 matmuls, 4 casts, one 512KB DMA (fewer dispatches/sems).
                # tail=True (final t-tile): split casts across DVE+ACT and
                # DMA per quarter on alternating queues to shorten the
                # epilogue critical chain.
                po = xp.tile([128, 4, 512], F16, tag="po", name="po", bufs=3)
                for quar in range(4):
                    e8 = e2 * 4 + quar
                    pp = ps.tile([128, 512], F32, tag="sm", name="pp", bufs=2)
                    nc.tensor.matmul(pp[:],
                                     wp_t[:, e8 * 128:(e8 + 1) * 128],
                                     headT[:, t * 512:t * 512 + 512],
                                     start=True, stop=True)
                    if tail and quar % 2 == 1:
                        nc.scalar.copy(po[:, quar, :], pp[:])
                    else:
                        nc.vector.tensor_copy(po[:, quar, :], pp[:])
                    if tail:
                        eng = nc.sync if quar % 2 == 0 else nc.scalar
                        eng.dma_start(
                            outT_d[e8 * 128:(e8 + 1) * 128,
                                   t * 512:t * 512 + 512].rearrange(
                                "(one p) t -> p one t", p=128),
                            po[:, quar:quar + 1, :])
                if not tail:
                    nc.sync.dma_start(
                        outT_d[e2 * 512:(e2 + 1) * 512,
                               t * 512:t * 512 + 512].rearrange(
                            "(four p) t -> p four t", p=128),
                        po[:])

            def emit_attn(fillers, pe_bcast_last=True):
                """attention for BOTH batches as one continuous 128-group
                stream; fillers: iterator of callables used to keep PE dense
                while ACT works through the exps"""
                # Software-pipelined over ALL (b, tq, tk) tiles: the
                # scores/exp stream runs 2 slots ahead of the attnv stream
                # continuously across tq AND batch boundaries, so the
                # pipeline never drains mid-kernel (a drain also triggered
                # HAM re-throttles).
                NTOT = B * TQ * TK
                ous = {}
                epipe = {}
                for g in range(NTOT + 2):
                    # scores go FIRST in the slot: at tq boundaries the attnv
                    # stream blocks on the ou-ring until the norm evacuation
                    # frees it — with scores ahead in the PE FIFO, the
                    # exp pipeline keeps flowing through the stall.
                    if g < NTOT:
                        b, rem = divmod(g, TQ * TK)
                        tq, tk = divmod(rem, TK)
                        q0 = b * L + tq * 512
                        k0 = b * L + tk * 128
                        s = ps.tile([128, 1024], F32, tag="sc", name="sB",
                                     bufs=2)
                        # high_priority pins the pair + exp at the head of
                        # the scheduler's ready heap: without it, the
                        # scheduler sometimes splits the tile-position pair
                        # around filler matmuls, delaying the exp ~2us
                        with tc.high_priority():
                            nc.tensor.matmul(s[:, 0:512],
                                             kTt[0:64, k0:k0 + 128],
                                             qT[0:64, q0:q0 + 512],
                                             start=True, stop=True,
                                             tile_position=(0, 0))
                            nc.tensor.matmul(s[:, 512:1024],
                                             kTt[64:128, k0:k0 + 128],
                                             qT[64:128, q0:q0 + 512],
                                             start=True, stop=True,
                                             tile_position=(64, 0))
                            e = ep.tile([128, 1024], F16, tag="e", name="e")
                            nc.scalar.activation(e[:], s[:], EXP)
                        epipe[g] = e
                    if fillers is not None:
                        try:
                            next(fillers)()
                        except StopIteration:
                            fillers = None
                    ga = g - 2
                    if ga < 0:
                        continue
                    b_a, rem_a = divmod(ga, TQ * TK)
                    aq, aj = divmod(rem_a, TK)
                    gq = ga // TK      # global tq index (0..B*TQ-1)
                    if aj == 0:
                        ous[gq] = [ps.tile([65, 512], F32, tag="outU",
                                           name=f"ou{h}", bufs=2)
                                   for h in range(2)]
                    ou = ous[gq]
                    e = epipe.pop(ga)
                    nc.tensor.matmul(ou[0][:], vblk[b_a][aj][:, 0:65],
                                     e[:, 0:512],
                                     start=(aj == 0), stop=(aj == TK - 1))
                    nc.tensor.matmul(ou[1][:], vblk[b_a][aj][:, 80:145],
                                     e[:, 512:1024],
                                     start=(aj == 0), stop=(aj == TK - 1))
                    if aj != TK - 1:
                        continue
                    # tq aq complete: evacuate ou to SBUF so the PSUM banks
                    # free quickly; normalization runs from SBUF off the
                    # critical path. ou = [A rows 0-63; Z row 64] per head.
                    # Z rows + reciprocals first so the gpsimd broadcasts
                    # start while the DVE still evacuates the A rows.
                    ou = ous.pop(gq)
                    qa = b_a * L + aq * 512
                    pe_bc = pe_bcast_last and ga == NTOT - 1
                    # The four ou evacuation copies (rs = Z rows, ouS = A
                    # rows) come FIRST and elevated: the ou ring — and with
                    # it the next tq's attnv — frees as soon as they run.
                    # The slow single-partition reciprocals (~0.7us each),
                    # broadcasts and muls follow off the critical path.
                    rss, ouSs = [], []
                    for h in range(2):
                        rs = np_.tile([1, 512], F32, tag=f"rs{h}",
                                      name=f"rs{h}")
                        if pe_bc:
                            # ACT is idle at stream end: off-load the copies
                            # so the DVE queue only carries recip+mul on the
                            # epilogue critical path
                            nc.scalar.copy(rs[:], ou[h][64:65, :])
                        else:
                            with tc.high_priority():
                                nc.vector.tensor_copy(rs[:], ou[h][64:65, :])
                        rss.append(rs)
                    for h in range(2):
                        ouS = np_.tile([64, 512], F32, tag=f"ouS{h}",
                                       name=f"ouS{h}")
                        if pe_bc:
                            nc.scalar.copy(ouS[:], ou[h][0:64, :])
                        else:
                            with tc.high_priority():
                                nc.vector.tensor_copy(ouS[:], ou[h][0:64, :])
                        ouSs.append(ouS)
                    rrs = []
                    for h in range(2):
                        r = np_.tile([1, 512], F32, tag=f"r{h}", name=f"r{h}")
                        nc.vector.reciprocal_approx_fast(r[:], rss[h])
                        if pe_bc:
                            # last tq of the run: broadcast 1/Z on the PE —
                            # the scores PSUM ring is free here, the gpsimd
                            # ucode broadcast (1.8us) sits on the epilogue
                            # critical path, and the matmul keeps HAM warm
                            # for the final proj
                            r16 = np_.tile([1, 512], F16, tag=f"r16{h}",
                                           name=f"r16{h}")
                            nc.scalar.copy(r16[:], r[:])
                            bcp = ps.tile([128, 1024], F32, tag="sc",
                                          name="bcp", bufs=2)
                            nc.tensor.matmul(bcp[0:64, 0:512], onesrow[:],
                                             r16[:], start=True, stop=True)
                            rrs.append(bcp[0:64, 0:512])
                        else:
                            bc = np_.tile([64, 512], F32, tag=f"bc{h}",
                                          name=f"bc{h}")
                            nc.gpsimd.partition_broadcast(bc[:], r[:])
                            rrs.append(bc[:])
                    for h in range(2):
                        nc.vector.tensor_mul(
                            headT[h * 64:(h + 1) * 64, qa:qa + 512],
                            ouSs[h], rrs[h])
                if fillers is not None:
                    for f in fillers:
                        f()

            def run_group(t, part):
                for _ in emit_A_group_gen(t, part):
                    pass

            def group_units2(t, part):
                # compressed 2-unit split (keeps the sA ring hold short and
                # the group's completion early)
                gen = emit_A_group_gen(t, part)
                return [lambda g=gen: next(g, None),
                        lambda g=gen: list(g)]

            def group_units3(t, part):
                gen = emit_A_group_gen(t, part)
                return [lambda g=gen: next(g, None),
                        lambda g=gen: next(g, None),
                        lambda g=gen: list(g)]

            def place(slots, at, units):
                for i, u in zip(at, units):
                    assert slots[i] is None, f"slot {i} already taken"
                    slots[i] = u

            with (tc.For_i(0, reps, 1) if reps > 1 else nullcontext()):
                # Minimal prologue: only what scores(0)/attnv(0) need (k, q,
                # v of t-tile 0). The first exp is then ~3 groups after the
                # first DMA lands instead of 5 — and none of the remaining
                # qkv evacuations sit ahead of it in the in-order DVE queue.
                for t, part in ((0, 1), (0, 0), (0, 2)):
                    run_group(t, part)
                # One unified 128-group attention stream over both batches.
                # Filler slot map (consumed at slot START; a group's last
                # unit must land before its consumer):
                #  b0: scores(tk) at slot tk reads kTt/qT, attnv(tk) at
                #      slot tk+2 reads vblk; qN by slot 16*N.
                #  b1: scores at slot 64+tk (kTt t4-7 map to b1 tk0-15),
                #      attnv at 66+tk; q4 by 64, q5 by 80, q6 by 96,
                #      q7 by 112.
                #  proj(t, e2) units go in PE-light odd slots well after the
                #  producing tq's norm, spread out so their DVE casts never
                #  chain (the sm ring serializes a bunched proj stream at
                #  cast rate).
                def fill():
                    def disp(ts):
                        for t in ts:
                            dma_x(t, 0, nc.sync)
                            dma_x(t, 1, nc.sync)
                    slots = [None] * 130
                    place(slots, (0, 1), group_units2(1, 1))
                    place(slots, (2, 3), group_units2(1, 2))
                    place(slots, (4, 6), group_units2(2, 1))
                    place(slots, (5, 7), group_units2(2, 2))
                    place(slots, (8, 9), group_units2(3, 1))
                    place(slots, (10, 11), group_units2(3, 2))
                    place(slots, (12, 13, 14), group_units3(1, 0))
                    place(slots, (16, 18, 20), group_units3(2, 0))
                    place(slots, (22, 24, 26), group_units3(3, 0))
                    place(slots, (28, 30, 32), group_units3(4, 1))
                    place(slots, (34, 36, 38), group_units3(4, 2))
                    place(slots, (40, 42, 44), group_units3(5, 1))
                    place(slots, (46, 48, 50), group_units3(5, 2))
                    place(slots, (52, 54, 56), group_units3(4, 0))
                    place(slots, (58, 60), group_units2(6, 1))
                    place(slots, (59, 61), group_units2(6, 2))
                    place(slots, (63, 65), group_units2(7, 1))
                    place(slots, (64, 66), group_units2(7, 2))
                    place(slots, (67, 70, 73), group_units3(5, 0))
                    place(slots, (77, 81, 85), group_units3(6, 0))
                    place(slots, (89, 93, 97), group_units3(7, 0))
                    proj_at = {0: (21, 25), 1: (37, 39), 2: (53, 55),
                               3: (75, 79), 4: (99, 103), 5: (107, 111),
                               6: (119, 123)}
                    for t, (a, b2) in proj_at.items():
                        place(slots, (a, b2),
                              [lambda tt=t: emit_proj(tt, 0),
                               lambda tt=t: emit_proj(tt, 1)])
                    extras = {0: lambda: disp((4, 5)), 1: lambda: disp((6, 7))}
                    for i, u in enumerate(slots):
                        e = extras.get(i)
                        if e is not None:
                            yield (lambda ee=e, uu=u:
                                   (ee(), uu() if uu else None) and None)
                        else:
                            yield u if u is not None else (lambda: None)
                emit_attn(fill())
                # epilogue: only the last t-tile's proj remains
                for e2 in range(2):
                    emit_proj(7, e2, tail=True)

    nc.compile()
    return nc

_CACHE = {}


def _get_nc(reps: int = 1):
    key = reps
    if key not in _CACHE:
        _CACHE[key] = _build_nc(reps)
    return _CACHE[key]


def _make_in_maps(x, w_qkv, w_proj):
    xT = np.ascontiguousarray(x.reshape(T, D).T).astype(np.float16)
    in_maps = []
    for c in range(NCORES):
        j0 = c * 128
        wq = w_qkv[j0:j0 + 128] * 0.125          # fold attention scale into q
        wk = w_qkv[D + j0:D + j0 + 128]
        wv = w_qkv[2 * D + j0:2 * D + j0 + 128]
        wqkvT = np.ascontiguousarray(
            np.concatenate([wq, wk, wv], axis=0).T).astype(np.float16)
        wprojT = np.ascontiguousarray(w_proj[:, j0:j0 + 128].T).astype(np.float16)
        in_maps.append({"xT": xT, "wqkvT": wqkvT, "wprojT": wprojT})
    return in_maps


def _numpy_reference(x, mask, w_qkv, w_proj):
    x64 = x.astype(np.float64)
    qkv = (x64 @ w_qkv.T.astype(np.float64)).reshape(B, L, 3, H, HEAD_DIM)
    qkv = qkv.transpose(2, 0, 3, 1, 4)
    q, k, v = qkv[0], qkv[1], qkv[2]
    attn = np.einsum('bhqd,bhkd->bhqk', q, k) * (HEAD_DIM ** -0.5)
    attn = np.where(mask[:, None, :, :], attn, -np.inf)
    attn = attn - attn.max(axis=-1, keepdims=True)
    attn = np.exp(attn)
    attn = attn / attn.sum(axis=-1, keepdims=True)
    out = np.einsum('bhqk,bhkd->bhqd', attn, v)
    out = out.transpose(0, 2, 1, 3).reshape(B, L, D)
    return (out @ w_proj.T.astype(np.float64)).astype(np.float32)


def kernel(x, mask, w_qkv, w_proj):
    x = np.asarray(x)
    mask = np.asarray(mask)
    w_qkv = np.asarray(w_qkv)
    w_proj = np.asarray(w_proj)
    if not mask.all():
        # spec guarantees an all-ones mask; keep a correct fallback anyway
        return _numpy_reference(x, mask, w_qkv, w_proj)

    from concourse import bass_utils
    nc = _get_nc()
    in_maps = _make_in_maps(x, w_qkv, w_proj)
    res = bass_utils.run_bass_kernel_spmd(nc, in_maps,
                                          core_ids=list(range(NCORES)))
    acc = np.zeros((D, T), np.float32)
    for c in range(NCORES):
        acc += res.results[c]["outT"].astype(np.float32)
    return np.ascontiguousarray(acc.T).reshape(B, L, D)


if __name__ == "__main__":
    rng = np.random.default_rng(0)
    x = rng.standard_normal((B, L, D)).astype(np.float32)
    mask = np.ones((B, L, L), bool)
    w_qkv = (rng.standard_normal((3 * D, D)) * D ** -0.5).astype(np.float32)
    w_proj = (rng.standard_normal((D, D)) * D ** -0.5).astype(np.float32)
    out = kernel(x, mask, w_qkv, w_proj)
    exp = _numpy_reference(x, mask, w_qkv, w_proj)
    err = np.abs(out - exp).max() / np.abs(exp).max()
    print("rel err vs fp64 numpy reference:", err)



# revision 40
# speedup vs baseline: 1.0101x; 1.0041x over previous
"""Multi-head attention (B=2, L=2048, D=1024, H=16, hd=64) on 8 TRN2 NeuronCores.

Sharding: tensor-parallel over heads — 2 heads per core. Each core computes
qkv projection for its heads, full attention for its (b, h) pairs, and a
partial output projection (w_proj columns for its heads); the host sums the
8 partial projections (f16 partials, accumulated in f32 host-side).

All matmuls are f16 operands with fp32 PSUM accumulation.

Dataflow per core (contraction dim always on partitions):
  qT,kT,vT [128, 4096] = w-slice.T @ xT        (transposed layout [j, t])
  vblk [128,160] per 128-key tile: [v_h0|1 .. v_h1|1] via PE transpose + copies
  scoresT  [tk, tq] = kT.T-slices @ qT-slices  (2 heads packed via tile_position)
  expT = exp(scoresT)                          (ACT, psum->sbuf f16)
  ou_h [65, tq] = [v_h|1].T @ expT             (fused attn@v + softmax denom)
  head = ou[v-rows] * bcast(1/ou[denom-row])   (DVE recip + GPSIMD bcast)
  outT_partial [e, t] = wprojT-slices.T @ headT  (f16 out, host sums partials)

Schedule: PE warm-up junk matmuls bridge the DMA prologue (HAM clock gate),
then a minimal prologue (k/q/v of t-tile 0 only), then ONE unified
128-group (b, tq, tk) attention stream with all remaining qkv groups and
all proj work as slot-mapped PE fillers; only the last t-tile's proj is in
the epilogue (with casts split DVE/ACT and per-quarter output DMAs).
Prologue DMAs are split across the sync+scalar HWDGE queues in
first-use order. Scores pairs + exps and the evacuation/normalization
DVE chain run under tc.high_priority() so the scheduler cannot park them
behind filler work; the e-ring is 8 deep so an exp's WAR on attnv(g-8)
never bites at tq boundaries. Filler units are consumed once per slot
(after the scores emission) and placed so every producer lands before its
consumer's slot, with proj units spread into PE-light odd slots well after
the producing tq's norm.
"""
import sys

if '/opt/trn_rl_repo' not in sys.path:
    sys.path.insert(0, '/opt/trn_rl_repo')

import numpy as np

B, L, D = 2, 2048, 1024
HEAD_DIM = 64
H = D // HEAD_DIM          # 16
NCORES = 8
HPC = H // NCORES          # 2 heads per core
T = B * L                  # 4096
KT = D // 128              # 8 contraction tiles for the projections
TT = T // 512              # 8 t-tiles of 512
TQ = L // 512              # 4 query tiles per batch
TK = L // 128              # 16 key tiles per batch


VTRANS_DMA = False


def _build_nc(reps: int = 1):
    import concourse.bacc as bacc
    import concourse.mybir as mybir
    import concourse.tile as tile
    from concourse.masks import make_identity
    from contextlib import nullcontext

    F32 = mybir.dt.float32
    F16 = mybir.dt.float16
    EXP = mybir.ActivationFunctionType.Exp

    nc = bacc.Bacc("TRN2", target_bir_lowering=False, debug=False,
                   num_devices=NCORES)
    xT_d = nc.dram_tensor("xT", [D, T], F16, kind="ExternalInput").ap()
    wqkvT_d = nc.dram_tensor("wqkvT", [D, 3 * 128], F16, kind="ExternalInput").ap()
    wprojT_d = nc.dram_tensor("wprojT", [128, D], F16, kind="ExternalInput").ap()
    outT_d = nc.dram_tensor("outT", [D, T], F16, kind="ExternalOutput").ap()

    with tile.TileContext(nc) as tc:
        with nc.allow_low_precision(reason="f16 matmul pipeline by design"), \
             tc.tile_pool(name="const", bufs=1) as cp, \
             tc.tile_pool(name="xt", bufs=2) as xp, \
             tc.tile_pool(name="exp", bufs=12) as ep, \
             tc.tile_pool(name="nrm", bufs=2) as np_, \
             tc.tile_pool(name="ps", bufs=2, space="PSUM") as ps:

            ones_f = cp.tile([128, 1], F32, tag="onesf")
            nc.gpsimd.memset(ones_f[:], 1.0)
            # f16 ones row [1, 64]: stationary operand of the PE broadcast
            # matmul (out[64, 512] = onesrow.T @ r16)
            onesrow = cp.tile([1, 64], F16, tag="onesrow")
            nc.gpsimd.memset(onesrow[:], 1.0)
            if not VTRANS_DMA:
                ident_f = cp.tile([128, 128], F32, tag="identf")
                make_identity(nc, ident_f[:])
                ident = cp.tile([128, 128], F16, tag="ident")
                nc.vector.tensor_copy(ident[:], ident_f[:])

            # PE warm-up: the HAM clock gate keeps the PE at 1.2 GHz until
            # ~3.4us of sustained activity. Junk matmuls bridge the DMA
            # prologue so the real stream starts (nearly) warm.
            junkw = cp.tile([128, 256], F16, tag="junkw")
            nc.gpsimd.memset(junkw[:], 0.0)

            # weights: three per-part tiles so the first matmul only waits on
            # its own 256KB slice
            w_p = [cp.tile([128, KT, 128], F16, tag=f"w{p}", name=f"w{p}")
                   for p in range(3)]

            def dma_w(p, c0, n=4):
                # weight transfers ride the scalar (ACT) HWDGE queue: ACT is
                # idle until the first scores land, and this keeps the sync
                # queue free for the x tiles. c0/n select a chunk range.
                nc.scalar.dma_start(
                    w_p[p][:, c0:c0 + n, :],
                    wqkvT_d[:, p * 128:(p + 1) * 128].rearrange(
                        "(k p) j -> p k j", p=128)[:, c0:c0 + n, :])
            wp_t = cp.tile([128, 1024], F16, tag="wp")

            # persistent activations
            qT = cp.tile([128, T], F16, tag="qT")
            kTt = cp.tile([128, T], F16, tag="kTt")
            vT = cp.tile([128, T], F16, tag="vT")
            headT = cp.tile([128, T], F16, tag="headT")
            # vblk layout (160 cols): [v_h0 0:64 | ones 64 | pad | v_h1
            # 80:144 | ones 144 | pad]; lhsT h0 = [:, 0:65], h1 = [:, 80:145],
            # both -> ou = [A rows 0-63; Z row 64]. Both v halves are written
            # by ONE DMA-XBAR transpose via a [128, 2, 64] strided view
            # (dim1 stride 80 cols = 160B, 32B-aligned).
            vblk = [[cp.tile([128, 160], F16, tag=f"vb{b}_{tk}",
                             name=f"vb{b}_{tk}") for tk in range(TK)]
                    for b in range(B)]
            for b in range(B):
                for tk in range(TK):
                    # one strided memset covers both ones columns (64, 144);
                    # gpsimd keeps this startup work off the DVE
                    oc = vblk[b][tk][:, 0:160].rearrange(
                        "p (two c) -> p two c", c=80)[:, :, 64:65]
                    nc.gpsimd.memset(oc, 1.0)

            # x: all tiles prefetched, two DMAs per tile (matching the two
            # 4-chunk matmul halves) so consumers wait only on their half and
            # the sync engine isn't clogged with dispatches
            xts = [xp.tile([128, KT, 512], F16, tag="xt", name=f"xt{t}",
                           bufs=8) for t in range(TT)]

            def dma_x(t, h, eng, n=4):
                c0 = int(4 * h)
                eng.dma_start(
                    xts[t][:, c0:c0 + n, :],
                    xT_d[:, t * 512:t * 512 + 512].rearrange(
                        "(k p) t -> p k t", p=128)[:, c0:c0 + n, :])

            # Prologue transfers split across the two HWDGE dispatchers:
            # weights on scalar, x on sync, both ordered to match the
            # prologue group order (k0, q0, v0). Early transfers ride HBM
            # contention from all 8 cores prefetching at once, so the very
            # first chunks are split finer to get the first matmul going.
            dma_w(1, 0, 2)        # wk chunks 0-1 (scalar queue)
            dma_x(0, 0, nc.sync, 2)   # xt0 chunks 0-1 (sync queue)
            dma_w(1, 2, 2)        # wk chunks 2-3
            dma_x(0, 0.5, nc.sync, 2)  # xt0 chunks 2-3
            dma_w(1, 4, 4)        # wk h1
            dma_x(0, 1, nc.sync)  # xt0 h1
            dma_w(0, 0, 4)        # wq h0
            dma_w(0, 4, 4)        # wq h1
            dma_w(2, 0, 4)        # wv h0
            dma_w(2, 4, 4)        # wv h1
            dma_x(1, 0, nc.sync)
            dma_x(1, 1, nc.sync)
            for t in (2, 3):
                dma_x(t, 0, nc.sync)
                dma_x(t, 1, nc.sync)
            nc.sync.dma_start(wp_t[:], wprojT_d[:, :])
            # x t4-7 dispatches are deferred into early phase-1 filler slots
            # (they are not needed until ~40us in).

            # warm the ACT exp table (queued on scalar after the w
            # dispatches; done well before the first real scores)
            actw = cp.tile([128, 1], F32, tag="actw")
            nc.scalar.activation(actw[:], ones_f[:], EXP)

            # PE warm-up stream: ~16 junk N=256 matmuls ~= 3.4us at the cold
            # 1.2 GHz clock, so the HAM un-throttles by the time the real
            # pipeline is flowing
            jp = ps.tile([128, 512], F32, tag="sm", name="jp", bufs=2)
            for _ in range(10):
                nc.tensor.matmul(jp[:, 0:256], junkw[:, 0:128],
                                 junkw[:, 0:256], start=True, stop=True)

            def emit_A_group_gen(t, part):
                """one qkv projection group: dest[:, t-tile] for q/k/v part;
                generator yielding mid-group for finer interleaving"""
                dest = (qT, kTt, vT)[part]
                xt = xts[t]
                s = ps.tile([128, 512], F32, tag="sm", name="sA", bufs=2)
                for half in range(2):
                    for k in range(4 * half, 4 * half + 4):
                        nc.tensor.matmul(
                            s[:],
                            w_p[part][:, k, :],
                            xt[:, k, :],
                            start=(k == 0), stop=(k == KT - 1))
                    yield None
                # the evacuation gates scores/attnv of later slots — keep it
                # ahead of bulk DVE work (proj casts) in the queue
                with tc.high_priority():
                    nc.vector.tensor_copy(dest[:, t * 512:t * 512 + 512],
                                          s[:])
                if part == 2:
                    # v landed: transpose this t-tile's 4 key blocks into
                    # vblk — via DMA XBAR (pure DMA, no PE slot) or PE
                    b, tq = divmod(t, TQ)
                    for i in range(4):
                        tk = tq * 4 + i
                        c0 = b * L + tk * 128
                        vb = vblk[b][tk]
                        if VTRANS_DMA:
                            # XBAR transpose: out partition j <- src free j,
                            # out free col <- src partition. One DMA per head
                            # (a strided multi-dim dst is treated as extra
                            # partition dims by dma_start_transpose, so the
                            # two 64-col halves need separate transfers).
                            nc.sync.dma_start(vb[:, 0:64],
                                              vT[0:64, c0:c0 + 128],
                                              transpose=True)
                            nc.sync.dma_start(vb[:, 80:144],
                                              vT[64:128, c0:c0 + 128],
                                              transpose=True)
                        else:
                            p32 = ps.tile([128, 512], F32, tag="sm",
                                          name="ptr", bufs=2)
                            pt = p32.bitcast(F16)
                            nc.tensor.transpose(pt[:, 0:128],
                                                vT[:, c0:c0 + 128], ident[:])
                            nc.vector.tensor_copy(vb[:, 0:64],
                                                  pt[0:128, 0:64])
                            nc.vector.tensor_copy(vb[:, 80:144],
                                                  pt[0:128, 64:128])

            def emit_proj(t, e2, tail=False):
                # one unit = quarter of the output rows for one t-tile:
                # 4 matmuls, 4 casts, one 512KB DMA (fewer dispatches/sems).
                # tail=True (final t-tile): split casts across DVE+ACT and
                # DMA per quarter on alternating queues to shorten the
                # epilogue critical chain.
                po = xp.tile([128, 4, 512], F16, tag="po", name="po", bufs=3)
                for quar in range(4):
                    e8 = e2 * 4 + quar
                    if tail and quar % 2 == 1:
                        # the scores pool is idle in the epilogue: borrowing
                        # it for alternate quarters doubles the effective pp
                        # ring so the matmuls stop gating on cast completion
                        pw = ps.tile([128, 1024], F32, tag="sc", name="ppw",
                                     bufs=2)
                        pp = pw[:, 0:512]
                    else:
                        pp = ps.tile([128, 512], F32, tag="sm", name="pp",
                                     bufs=2)[:]
                    nc.tensor.matmul(pp,
                                     wp_t[:, e8 * 128:(e8 + 1) * 128],
                                     headT[:, t * 512:t * 512 + 512],
                                     start=True, stop=True)
                    if tail and quar % 2 == 1:
                        nc.scalar.copy(po[:, quar, :], pp)
                    else:
                        nc.vector.tensor_copy(po[:, quar, :], pp)
                    if tail:
                        eng = nc.sync if quar % 2 == 0 else nc.scalar
                        eng.dma_start(
                            outT_d[e8 * 128:(e8 + 1) * 128,
                                   t * 512:t * 512 + 512].rearrange(
                                "(one p) t -> p one t", p=128),
                            po[:, quar:quar + 1, :])
                if not tail:
                    nc.sync.dma_start(
                        outT_d[e2 * 512:(e2 + 1) * 512,
                               t * 512:t * 512 + 512].rearrange(
                            "(four p) t -> p four t", p=128),
                        po[:])

            def emit_attn(fillers, pe_bcast_last=True):
                """attention for BOTH batches as one continuous 128-group
                stream; fillers: iterator of callables used to keep PE dense
                while ACT works through the exps"""
                # Software-pipelined over ALL (b, tq, tk) tiles: the
                # scores/exp stream runs 2 slots ahead of the attnv stream
                # continuously across tq AND batch boundaries, so the
                # pipeline never drains mid-kernel (a drain also triggered
                # HAM re-throttles).
                NTOT = B * TQ * TK
                ous = {}
                epipe = {}
                for g in range(NTOT + 2):
                    # scores go FIRST in the slot: at tq boundaries the attnv
                    # stream blocks on the ou-ring until the norm evacuation
                    # frees it — with scores ahead in the PE FIFO, the
                    # exp pipeline keeps flowing through the stall.
                    if g < NTOT:
                        b, rem = divmod(g, TQ * TK)
                        tq, tk = divmod(rem, TK)
                        q0 = b * L + tq * 512
                        k0 = b * L + tk * 128
                        s = ps.tile([128, 1024], F32, tag="sc", name="sB",
                                     bufs=2)
                        # high_priority pins the pair + exp at the head of
                        # the scheduler's ready heap: without it, the
                        # scheduler sometimes splits the tile-position pair
                        # around filler matmuls, delaying the exp ~2us
                        with tc.high_priority():
                            nc.tensor.matmul(s[:, 0:512],
                                             kTt[0:64, k0:k0 + 128],
                                             qT[0:64, q0:q0 + 512],
                                             start=True, stop=True,
                                             tile_position=(0, 0))
                            nc.tensor.matmul(s[:, 512:1024],
                                             kTt[64:128, k0:k0 + 128],
                                             qT[64:128, q0:q0 + 512],
                                             start=True, stop=True,
                                             tile_position=(64, 0))
                            e = ep.tile([128, 1024], F16, tag="e", name="e")
                            nc.scalar.activation(e[:], s[:], EXP)
                        epipe[g] = e
                    if fillers is not None:
                        try:
                            next(fillers)()
                        except StopIteration:
                            fillers = None
                    ga = g - 2
                    if ga < 0:
                        continue
                    b_a, rem_a = divmod(ga, TQ * TK)
                    aq, aj = divmod(rem_a, TK)
                    gq = ga // TK      # global tq index (0..B*TQ-1)
                    if aj == 0:
                        ous[gq] = [ps.tile([65, 512], F32, tag="outU",
                                           name=f"ou{h}", bufs=2)
                                   for h in range(2)]
                    ou = ous[gq]
                    e = epipe.pop(ga)
                    nc.tensor.matmul(ou[0][:], vblk[b_a][aj][:, 0:65],
                                     e[:, 0:512],
                                     start=(aj == 0), stop=(aj == TK - 1))
                    nc.tensor.matmul(ou[1][:], vblk[b_a][aj][:, 80:145],
                                     e[:, 512:1024],
                                     start=(aj == 0), stop=(aj == TK - 1))
                    if aj != TK - 1:
                        continue
                    # tq aq complete: evacuate ou to SBUF so the PSUM banks
                    # free quickly; normalization runs from SBUF off the
                    # critical path. ou = [A rows 0-63; Z row 64] per head.
                    # Z rows + reciprocals first so the gpsimd broadcasts
                    # start while the DVE still evacuates the A rows.
                    ou = ous.pop(gq)
                    qa = b_a * L + aq * 512
                    pe_bc = pe_bcast_last and ga == NTOT - 1
                    # The four ou evacuation copies (rs = Z rows, ouS = A
                    # rows) come FIRST and elevated: the ou ring — and with
                    # it the next tq's attnv — frees as soon as they run.
                    # The slow single-partition reciprocals (~0.7us each),
                    # broadcasts and muls follow off the critical path.
                    rss, ouSs = [], []
                    for h in range(2):
                        rs = np_.tile([1, 512], F32, tag=f"rs{h}",
                                      name=f"rs{h}")
                        # always DVE: a [1,512] copy is ~190ns there vs
                        # ~680ns on ACT, and it heads the recip chain
                        with tc.high_priority():
                            nc.vector.tensor_copy(rs[:], ou[h][64:65, :])
                        rss.append(rs)
                    for h in range(2):
                        ouS = np_.tile([64, 512], F32, tag=f"ouS{h}",
                                       name=f"ouS{h}")
                        if pe_bc:
                            nc.scalar.copy(ouS[:], ou[h][0:64, :])
                        else:
                            with tc.high_priority():
                                nc.vector.tensor_copy(ouS[:], ou[h][0:64, :])
                        ouSs.append(ouS)
                    rrs = []
                    for h in range(2):
                        r = np_.tile([1, 512], F32, tag=f"r{h}", name=f"r{h}")
                        nc.vector.reciprocal_approx_fast(r[:], rss[h])
                        if pe_bc:
                            # last tq of the run: broadcast 1/Z on the PE —
                            # the scores PSUM ring is free here, the gpsimd
                            # ucode broadcast (1.8us) sits on the epilogue
                            # critical path, and the matmul keeps HAM warm
                            # for the final proj
                            r16 = np_.tile([1, 512], F16, tag=f"r16{h}",
                                           name=f"r16{h}")
                            nc.scalar.copy(r16[:], r[:])
                            bcp = ps.tile([128, 1024], F32, tag="sc",
                                          name="bcp", bufs=2)
                            nc.tensor.matmul(bcp[0:64, 0:512], onesrow[:],
                                             r16[:], start=True, stop=True)
                            rrs.append(bcp[0:64, 0:512])
                        else:
                            bc = np_.tile([64, 512], F32, tag=f"bc{h}",
                                          name=f"bc{h}")
                            nc.gpsimd.partition_broadcast(bc[:], r[:])
                            rrs.append(bc[:])
                    for h in range(2):
                        nc.vector.tensor_mul(
                            headT[h * 64:(h + 1) * 64, qa:qa + 512],
                            ouSs[h], rrs[h])
                if fillers is not None:
                    for f in fillers:
                        f()

            def run_group(t, part):
                for _ in emit_A_group_gen(t, part):
                    pass

            def group_units2(t, part):
                # compressed 2-unit split (keeps the sA ring hold short and
                # the group's completion early)
                gen = emit_A_group_gen(t, part)
                return [lambda g=gen: next(g, None),
                        lambda g=gen: list(g)]

            def group_units3(t, part):
                gen = emit_A_group_gen(t, part)
                return [lambda g=gen: next(g, None),
                        lambda g=gen: next(g, None),
                        lambda g=gen: list(g)]

            def place(slots, at, units):
                for i, u in zip(at, units):
                    assert slots[i] is None, f"slot {i} already taken"
                    slots[i] = u

            with (tc.For_i(0, reps, 1) if reps > 1 else nullcontext()):
                # Minimal prologue: only what scores(0)/attnv(0) need (k, q,
                # v of t-tile 0). The first exp is then ~3 groups after the
                # first DMA lands instead of 5 — and none of the remaining
                # qkv evacuations sit ahead of it in the in-order DVE queue.
                for t, part in ((0, 1), (0, 0), (0, 2)):
                    run_group(t, part)
                # One unified 128-group attention stream over both batches.
                # Filler slot map (consumed at slot START; a group's last
                # unit must land before its consumer):
                #  b0: scores(tk) at slot tk reads kTt/qT, attnv(tk) at
                #      slot tk+2 reads vblk; qN by slot 16*N.
                #  b1: scores at slot 64+tk (kTt t4-7 map to b1 tk0-15),
                #      attnv at 66+tk; q4 by 64, q5 by 80, q6 by 96,
                #      q7 by 112.
                #  proj(t, e2) units go in PE-light odd slots well after the
                #  producing tq's norm, spread out so their DVE casts never
                #  chain (the sm ring serializes a bunched proj stream at
                #  cast rate).
                def fill():
                    def disp(ts):
                        for t in ts:
                            dma_x(t, 0, nc.sync)
                            dma_x(t, 1, nc.sync)
                    slots = [None] * 130
                    place(slots, (0, 1), group_units2(1, 1))
                    place(slots, (2, 3), group_units2(1, 2))
                    place(slots, (4, 6), group_units2(2, 1))
                    place(slots, (5, 7), group_units2(2, 2))
                    place(slots, (8, 9), group_units2(3, 1))
                    place(slots, (10, 11), group_units2(3, 2))
                    place(slots, (12, 13, 14), group_units3(1, 0))
                    place(slots, (16, 18, 20), group_units3(2, 0))
                    place(slots, (22, 24, 26), group_units3(3, 0))
                    place(slots, (28, 30, 32), group_units3(4, 1))
                    place(slots, (34, 36, 38), group_units3(4, 2))
                    place(slots, (40, 42, 44), group_units3(5, 1))
                    place(slots, (46, 48, 50), group_units3(5, 2))
                    place(slots, (52, 54, 56), group_units3(4, 0))
                    place(slots, (58, 60), group_units2(6, 1))
                    place(slots, (59, 61), group_units2(6, 2))
                    place(slots, (63, 65), group_units2(7, 1))
                    place(slots, (64, 66), group_units2(7, 2))
                    place(slots, (67, 70, 73), group_units3(5, 0))
                    place(slots, (77, 81, 85), group_units3(6, 0))
                    place(slots, (89, 93, 97), group_units3(7, 0))
                    proj_at = {0: (21, 25), 1: (37, 39), 2: (53, 55),
                               3: (75, 79), 4: (99, 103), 5: (107, 111),
                               6: (119, 123)}
                    for t, (a, b2) in proj_at.items():
                        place(slots, (a, b2),
                              [lambda tt=t: emit_proj(tt, 0),
                               lambda tt=t: emit_proj(tt, 1)])
                    extras = {0: lambda: disp((4, 5)), 1: lambda: disp((6, 7))}
                    for i, u in enumerate(slots):
                        e = extras.get(i)
                        if e is not None:
                            yield (lambda ee=e, uu=u:
                                   (ee(), uu() if uu else None) and None)
                        else:
                            yield u if u is not None else (lambda: None)
                emit_attn(fill())
                # epilogue: only the last t-tile's proj remains
                for e2 in range(2):
                    emit_proj(7, e2, tail=True)

    nc.compile()
    return nc

_CACHE = {}


def _get_nc(reps: int = 1):
    key = reps
    if key not in _CACHE:
        _CACHE[key] = _build_nc(reps)
    return _CACHE[key]


def _make_in_maps(x, w_qkv, w_proj):
    xT = np.ascontiguousarray(x.reshape(T, D).T).astype(np.float16)
    in_maps = []
    for c in range(NCORES):
        j0 = c * 128
        wq = w_qkv[j0:j0 + 128] * 0.125          # fold attention scale into q
        wk = w_qkv[D + j0:D + j0 + 128]
        wv = w_qkv[2 * D + j0:2 * D + j0 + 128]
        wqkvT = np.ascontiguousarray(
            np.concatenate([wq, wk, wv], axis=0).T).astype(np.float16)
        wprojT = np.ascontiguousarray(w_proj[:, j0:j0 + 128].T).astype(np.float16)
        in_maps.append({"xT": xT, "wqkvT": wqkvT, "wprojT": wprojT})
    return in_maps


def _numpy_reference(x, mask, w_qkv, w_proj):
    x64 = x.astype(np.float64)
    qkv = (x64 @ w_qkv.T.astype(np.float64)).reshape(B, L, 3, H, HEAD_DIM)
    qkv = qkv.transpose(2, 0, 3, 1, 4)
    q, k, v = qkv[0], qkv[1], qkv[2]
    attn = np.einsum('bhqd,bhkd->bhqk', q, k) * (HEAD_DIM ** -0.5)
    attn = np.where(mask[:, None, :, :], attn, -np.inf)
    attn = attn - attn.max(axis=-1, keepdims=True)
    attn = np.exp(attn)
    attn = attn / attn.sum(axis=-1, keepdims=True)
    out = np.einsum('bhqk,bhkd->bhqd', attn, v)
    out = out.transpose(0, 2, 1, 3).reshape(B, L, D)
    return (out @ w_proj.T.astype(np.float64)).astype(np.float32)


def kernel(x, mask, w_qkv, w_proj):
    x = np.asarray(x)
    mask = np.asarray(mask)
    w_qkv = np.asarray(w_qkv)
    w_proj = np.asarray(w_proj)
    if not mask.all():
        # spec guarantees an all-ones mask; keep a correct fallback anyway
        return _numpy_reference(x, mask, w_qkv, w_proj)

    from concourse import bass_utils
    nc = _get_nc()
    in_maps = _make_in_maps(x, w_qkv, w_proj)
    res = bass_utils.run_bass_kernel_spmd(nc, in_maps,
                                          core_ids=list(range(NCORES)))
    acc = np.zeros((D, T), np.float32)
    for c in range(NCORES):
        acc += res.results[c]["outT"].astype(np.float32)
    return np.ascontiguousarray(acc.T).reshape(B, L, D)


if __name__ == "__main__":
    rng = np.random.default_rng(0)
    x = rng.standard_normal((B, L, D)).astype(np.float32)
    mask = np.ones((B, L, L), bool)
    w_qkv = (rng.standard_normal((3 * D, D)) * D ** -0.5).astype(np.float32)
    w_proj = (rng.standard_normal((D, D)) * D ** -0.5).astype(np.float32)
    out = kernel(x, mask, w_qkv, w_proj)
    exp = _numpy_reference(x, mask, w_qkv, w_proj)
    err = np.abs(out - exp).max() / np.abs(exp).max()
    print("rel err vs fp64 numpy reference:", err)

